# revision 35
# baseline (speedup 1.0000x reference)
"""Distributed Trainium2 kernel for a 16-head self-attention block.

Model (reference):
    qkv = x @ W_qkv + b_qkv ; q,k,v per 16 heads (head_dim 64)
    attn = softmax(q k^T / 8) ; out = (attn @ v heads concat) @ W_out + b_out
Shapes: x [2, 2048, 1024], W_qkv [1024, 3072], W_out [1024, 1024].

Sharding (8 NeuronCores): 2 batch groups x 4 cores; each core owns one batch
element and 4 of the 16 heads (Megatron-style column-parallel QKV + row-
parallel out-proj). Each core computes its partial out-projection
[2048, 1024]; the host sums the 4 partials per batch (the Megatron
all-reduce, performed at unshard time) and adds the output bias.

Numerical notes:
  * softmax runs without max-subtraction: scores/8 here are bounded ~|2.5|.
  * the V bias folds out of attention exactly (softmax rows sum to 1), so
    bv@W_out is added to the host-side output bias instead of on device.

Device dataflow per core (no transposes anywhere):
  x^T [1024, 2048] (host pre-transposed, DMA'd in 512-column chunks so the
  first projection group starts ~5us in) ->
  Q^T,K^T [256, 2048] = Wq^T x^T (+bias on eviction);  V [2048, 4, 65]
    (per-head 64 dims + a ones column, which makes the PV matmul emit the
    softmax denominator as row 64 of U^T)
  attention in 8 blocks of (head-pair t, 512-query quarter qq):
    per 128-key tile kt ONE [128 k, 1024] PSUM tile holds both heads'
    scores^T (A cols 0:512, B cols 512:1024; the two 64-contract matmuls
    are row-tiles T0/T8), ONE 1024-wide exp on ScalarE -> E [128, 1024]
    bf16; pv [65, 512] per head += V[kt]^T E-half, emitted one kt behind
    so the PV matmuls are never semaphore-gated.
  The single exp per kt frees both score slots at once, so the next kt's
  pair of score matmuls gate on one event (sc ring bufs=3, 6 PSUM banks;
  pv pair 2 banks; 8 total).
  normalization (all off ScalarE — the old Ln/Exp route thrashed the
  activation-table sets): pv evicted to SBUF (DVE), denominator row DMA'd
  to a partition-0 tile (the custom-DVE reciprocal misreads nonzero input
  base partitions), reciprocal_approx_fast (DVE), GpSimd partition
  broadcast, one DVE multiply into ut (bf16).
  out partial [2048, 1024] bf16 = (U^T)^T @ Wo_rows, per 128-row slice.

Weave schedule (keeps ScalarE's exp stream as the pacer):
  pre-phase: K0 (all 4 qb) + Q0 qb0 only (~9us of PE), so the first exp
  fires ~15us in.
  block (t=0,qq=0): V projection (per-kt, just-in-time for PV) + Q0 qb1.
  blocks (0,1..3): K1/Q1 groups (1 per 3 kts) + Q0 qb2/qb3.
  blocks (1,1..3): out-projection of query quarter qq-1 (4 slices each,
  delayed to kt 4.. so the previous block's normalize chain never
  head-of-line blocks the PE FIFO).
  tail: PE filler matmuls bridge the last normalize chain (keeps the HAM
  clock-gate at 8/8), then the last quarter's out-projection.
"""

import contextlib
import os

import numpy as np

import concourse.bacc as bacc
import concourse.mybir as mybir
import concourse.tile as tile
from concourse import bass_utils

F32 = mybir.dt.float32
AF = mybir.ActivationFunctionType

S = 2048          # sequence length (one batch element per core)
E = 1024          # embedding dim
HD = 64           # head dim
NH = 4            # heads per core
DQ = NH * HD      # per-core q/k/v width (256)
ET = E // 128     # embedding tiles (8)
ST = S // 128     # sequence tiles (16)
QB = S // 512     # 512-wide query blocks (4)

MODE = os.environ.get("ATTN_MM_DTYPE", "bf16")  # bf16 | f32r | f32

_CACHED = {}


def build_core_graph(mode=MODE):
    MD = {"f32r": mybir.dt.float32r, "f32": F32, "bf16": mybir.dt.bfloat16}[mode]

    nc = bacc.Bacc("TRN2", target_bir_lowering=False, debug=False, num_devices=8)

    xt_d = nc.dram_tensor("xt", [E, S], MD, kind="ExternalInput")
    wq_d = nc.dram_tensor("wq", [E, DQ], MD, kind="ExternalInput")
    wk_d = nc.dram_tensor("wk", [E, DQ], MD, kind="ExternalInput")
    wv_d = nc.dram_tensor("wv", [E, DQ], MD, kind="ExternalInput")
    bq_d = nc.dram_tensor("bq", [DQ, 1], F32, kind="ExternalInput")
    bk_d = nc.dram_tensor("bk", [DQ, 1], F32, kind="ExternalInput")
    wo_d = nc.dram_tensor("wo", [DQ, E], MD, kind="ExternalInput")
    out_d = nc.dram_tensor("out", [S, E], mybir.dt.bfloat16, kind="ExternalOutput")

    with tile.TileContext(nc) as tc:
        with contextlib.ExitStack() as ctx:
            # ---- persistent SBUF pools ------------------------------------
            pers = ctx.enter_context(tc.tile_pool(name="pers", bufs=1))

            def ptile(shape, dtype, nm):
                return pers.tile(shape, dtype, tag=nm, name=nm)

            qt = [ptile([128, S], MD, f"qt{t}") for t in range(2)]
            ones_c = ptile([128, NH], F32, "ones_c")
            kt_sb = [ptile([128, S], MD, f"kt{t}") for t in range(2)]
            v_sb = [ptile([128, NH, HD + 1], MD, f"v{st}") for st in range(ST)]
            ut = [ptile([128, S], MD, f"ut{t}") for t in range(2)]
            wo_sb = [ptile([128, E], MD, f"wo{t}") for t in range(2)]
            bq_sb = [ptile([128, 1], F32, f"bq{t}") for t in range(2)]
            bk_sb = [ptile([128, 1], F32, f"bk{t}") for t in range(2)]

            nc.vector.memset(ones_c[:], 1.0)
            # dummy exp at t=0: hoists the one exp ACT_TABLE_LOAD into the
            # input-DMA window instead of right before the first real exp.
            dummy = ptile([1, 8], F32, "dummy")
            nc.vector.memset(dummy[:], 0.0)
            nc.scalar.activation(dummy[:], dummy[:], AF.Exp)
            # the ones column of every V tile is constant — write once here
            for st in range(ST):
                nc.vector.tensor_copy(v_sb[st][:, :, HD:HD + 1],
                                      ones_c[:].rearrange("p (h d) -> p h d", h=NH))
            for t in range(2):
                nc.scalar.dma_start(bq_sb[t][:], bq_d[t * 128:(t + 1) * 128, :])
                nc.scalar.dma_start(bk_sb[t][:], bk_d[t * 128:(t + 1) * 128, :])

            # x^T and weight tiles live until the last woven projection is
            # done. x^T is chunked per 512-query block so the first K
            # projection group only waits for ~1.5MB of input.
            apool = ctx.enter_context(tc.tile_pool(name="ab_sbuf", bufs=1))

            def atile(shape, nm):
                return apool.tile(shape, MD, tag=nm, name=nm)

            xtq = [[atile([128, 512], f"xt{et}_{qb}") for et in range(ET)]
                   for qb in range(QB)]
            # K/Q weights split per head-pair: the pre-phase needs only pair
            # 0's columns, so pair 1's 0.5MB stays off the critical input
            # stream until after the x^T chunks.
            wqs = [[atile([128, 128], f"wq{et}_{t}") for et in range(ET)]
                   for t in range(2)]
            wks = [[atile([128, 128], f"wk{et}_{t}") for et in range(ET)]
                   for t in range(2)]
            wvs = [atile([128, DQ], f"wv{et}") for et in range(ET)]
            # Input DMA: the projections consume x^T at ~590GB/s of demand,
            # so the head is input-bandwidth-bound. One issuing engine only
            # reaches its own few HWDGE queues (~130GB/s observed); stripe
            # every transfer round-robin across all three DMA-capable
            # engines (SP + ACT hwdge, Pool swdge) to engage the full queue
            # set. Order = first-consumption order.
            # The input demand curve (~2.5MB by 10us, +1MB per ~5us after)
            # exactly matches the measured ~250GB/s contended per-core HBM
            # rate, but ONLY if transfers complete in consumption order
            # globally. Per-engine queue FIFOs are imbalanced, so stripe one
            # priority-ordered list across the engines weighted by their
            # queue capacity (gpsimd SWDGE ~8 queues : sync ~6 : scalar ~4).
            def _sl(et):
                return slice(et * 128, (et + 1) * 128)

            dma_list = []
            for et in range(ET):
                dma_list.append((wks[0][et][:], wk_d[_sl(et), 0:128]))
                dma_list.append((xtq[0][et][:], xt_d[_sl(et), 0:512]))
            for et in range(ET):
                dma_list.append((wqs[0][et][:], wq_d[_sl(et), 0:128]))
            for et in range(ET):
                dma_list.append((wvs[et][:], wv_d[_sl(et), :]))
            for qb in range(1, QB):
                qsl = slice(qb * 512, (qb + 1) * 512)
                for et in range(ET):
                    dma_list.append((xtq[qb][et][:], xt_d[_sl(et), qsl]))
            for et in range(ET):
                dma_list.append((wks[1][et][:], wk_d[_sl(et), 128:256]))
                dma_list.append((wqs[1][et][:], wq_d[_sl(et), 128:256]))
            for t in range(2):
                dma_list.append((wo_sb[t][:], wo_d[t * 128:(t + 1) * 128, :]))
            pat = [nc.gpsimd, nc.sync, nc.scalar, nc.gpsimd, nc.sync,
                   nc.gpsimd, nc.scalar, nc.sync, nc.gpsimd]
            for i, (dst, src) in enumerate(dma_list):
                pat[i % len(pat)].dma_start(dst, src)

            def proj_group(dst, wsrc, tsl, qb, bias, psum_pool):
                qsl = slice(qb * 512, (qb + 1) * 512)
                pp = psum_pool.tile([128, 512], F32, tag="sc", name="sc")
                for et in range(ET):
                    nc.tensor.matmul(pp[:], wsrc[et][:, tsl], xtq[qb][et][:],
                                     start=(et == 0), stop=(et == ET - 1))
                nc.vector.tensor_scalar_add(dst[:, qsl], pp[:], bias[:])

            def v_unit(kt, psum_pool):
                # V projection for one 128-row sequence tile: reuses the
                # x^T chunks already resident for the K/Q groups.
                ssl = slice((kt % 4) * 128, (kt % 4 + 1) * 128)
                pv = psum_pool.tile([128, DQ], F32, tag="sc", name="sc")
                for et in range(ET):
                    nc.tensor.matmul(pv[:], xtq[kt // 4][et][:, ssl],
                                     wvs[et][:],
                                     start=(et == 0), stop=(et == ET - 1))
                nc.vector.tensor_copy(v_sb[kt][:, :, 0:HD],
                                      pv[:].rearrange("p (h d) -> p h d", h=NH))

            # ---- stage A-pre: warmup + K0 qb0 + Q0 qb0 + V 0..3 -----------
            # Minimal: just enough for block (0,0)'s first 4 key tiles. The
            # rest of K0 and V weave into block (0,0) just-in-time as their
            # x^T chunks land (input streaming is ~250GB/s, so the head is
            # bandwidth-gated; the exp stream must start as early as
            # possible).
            with tc.tile_pool(name="a_ps", bufs=5, space="PSUM") as a_ps:
                # HAM warm-up: keep the PE busy during the input-DMA window
                # so the clock-gate releases (K=8/8) before the real matmul
                # stream begins. Garbage data, never read.
                warm_sb = apool.tile([128, 512], MD, tag="warm", name="warm")
                nc.vector.memset(warm_sb[:], 0.0)
                wps = a_ps.tile([128, 512], F32, tag="wps", name="wps", bufs=1)
                for _ in range(12):
                    nc.tensor.matmul(wps[:], warm_sb[:, 0:128], warm_sb[:],
                                     start=True, stop=True)

                proj_group(kt_sb[0], wks[0], slice(0, 128), 0, bk_sb[0], a_ps)
                proj_group(qt[0], wqs[0], slice(0, 128), 0, bq_sb[0], a_ps)
                for kt in range(4):
                    v_unit(kt, a_ps)

            # ---- stage B: attention in 8 (head-pair, query-quarter) blocks
            with tc.tile_pool(name="e_sb", bufs=6) as e_pool, \
                 tc.tile_pool(name="uc_sb", bufs=4) as uc_pool, \
                 tc.tile_pool(name="rc_sb", bufs=4) as rc_pool, \
                 tc.tile_pool(name="rcb_sb", bufs=2) as rcb_pool, \
                 tc.tile_pool(name="o_sb", bufs=3) as o_pool, \
                 tc.tile_pool(name="sc_ps", bufs=3, space="PSUM") as sc_ps, \
                 tc.tile_pool(name="pv_ps", bufs=1, space="PSUM") as pv_ps:

                def out_proj(st, evict_scalar=False):
                    ssl = slice(st * 128, (st + 1) * 128)
                    op = sc_ps.tile([128, E], F32, tag="sc", name="sc")
                    for ob in range(2):
                        osl = slice(ob * 512, (ob + 1) * 512)
                        for t2 in range(2):
                            nc.tensor.matmul(op[:, osl], ut[t2][:, ssl],
                                             wo_sb[t2][:, osl],
                                             start=(t2 == 0), stop=(t2 == 1))
                    o_sb = o_pool.tile([128, E], mybir.dt.bfloat16, tag="o", name="o")
                    if evict_scalar:
                        # tail-only: ScalarE is idle there, and alternating
                        # the evictions doubles PSUM-drain throughput
                        nc.scalar.activation(o_sb[:], op[:], AF.Copy)
                    else:
                        nc.vector.tensor_copy(o_sb[:], op[:])
                    # two half-width DMAs land on different queues: halves
                    # the post-eviction transfer latency on the tail slices.
                    nc.sync.dma_start(out_d[ssl, 0:512], o_sb[:, 0:512])
                    nc.sync.dma_start(out_d[ssl, 512:1024], o_sb[:, 512:1024])

                # block (0,0) weave: the rest of K0 and V, just-in-time.
                # V(j) is consumed by emit_pv(j), which executes at kt j+1;
                # weaving V(j) at kt j is safe (the weave precedes scores
                # and the lagging PV in the PE FIFO). K0 qb g is first read
                # by scores kt=4g; woven 3-4 kts ahead. Both match the x^T
                # chunk arrival order.
                # kt -> half-group; V units fill the rest just-in-time.
                b00_k0 = {1: ("k0_1", 0), 2: ("k0_1", 1),
                          5: ("k0_2", 0), 6: ("k0_2", 1),
                          9: ("k0_3", 0), 10: ("k0_3", 1)}

                def weave_b00(kt):
                    if kt == 0:
                        proj_group(qt[0], wqs[0], slice(0, 128), 1, bq_sb[0],
                                   sc_ps)
                    g = b00_k0.get(kt)
                    if g is not None:
                        proj_half(*g)
                    if 2 <= kt <= 4:
                        v_unit(kt + 2, sc_ps)
                    elif 6 <= kt <= 8:
                        v_unit(kt + 1, sc_ps)
                    elif kt >= 10:
                        v_unit(kt, sc_ps)

                # remaining projection groups, identified by name, woven into
                # specific (block, kt) slots below. Deadlines: Q0qbN before
                # block (0,N); K1qb0+Q1qb0 before block (1,0); K1qbN before
                # block (1,0) kt 4N; Q1qbN before block (1,N).
                G = {}
                for qb in range(1, QB):
                    G[f"q0_{qb}"] = (qt[0], wqs[0], slice(0, 128), qb, bq_sb[0])
                    G[f"k0_{qb}"] = (kt_sb[0], wks[0], slice(0, 128), qb,
                                     bk_sb[0])
                for qb in range(QB):
                    G[f"k1_{qb}"] = (kt_sb[1], wks[1], slice(0, 128), qb,
                                     bk_sb[1])
                    G[f"q1_{qb}"] = (qt[1], wqs[1], slice(0, 128), qb,
                                     bq_sb[1])

                held = {}

                def proj_half(gname, half):
                    # half a projection group (4 of 8 contract tiles): keeps
                    # the per-kt PE overload under ~0.9us so the exp stream
                    # bubbles stay small. The PSUM accumulator tile is held
                    # across the two halves (intervening matmuls hit other
                    # banks — legal).
                    dst, wsrc, tsl, qb, bias = G[gname]
                    qsl = slice(qb * 512, (qb + 1) * 512)
                    if half == 0:
                        pp = sc_ps.tile([128, 512], F32, tag="sc", name="sc")
                        held[gname] = pp
                        for et in range(4):
                            nc.tensor.matmul(pp[:], wsrc[et][:, tsl],
                                             xtq[qb][et][:],
                                             start=(et == 0), stop=False)
                    else:
                        pp = held.pop(gname)
                        for et in range(4, ET):
                            nc.tensor.matmul(pp[:], wsrc[et][:, tsl],
                                             xtq[qb][et][:],
                                             start=False, stop=(et == ET - 1))
                        nc.vector.tensor_scalar_add(dst[:, qsl], pp[:],
                                                    bias[:])

                def weave_sched(sched):
                    def w(kt):
                        for gname, half in sched.get(kt, ()):
                            proj_half(gname, half)
                    return w

                def attn_block(t, qq, weave):
                    hA, hB = 2 * t, 2 * t + 1
                    pslA, pslB = slice(0, 64), slice(64, 128)
                    qsl = slice(qq * 512, (qq + 1) * 512)
                    pvpA = pv_ps.tile([HD + 1, 512], F32, tag="pvA", name="pvA")
                    pvpB = pv_ps.tile([HD + 1, 512], F32, tag="pvB", name="pvB")

                    def emit_pv(e, kt):
                        nc.tensor.matmul(pvpA[:], v_sb[kt][:, hA, :],
                                         e[:, 0:512],
                                         start=(kt == 0), stop=(kt == ST - 1))
                        nc.tensor.matmul(pvpB[:], v_sb[kt][:, hB, :],
                                         e[:, 512:1024],
                                         start=(kt == 0), stop=(kt == ST - 1))

                    pending = None
                    for kt in range(ST):
                        if weave is not None:
                            weave(kt)
                        ksl = slice(kt * 128, (kt + 1) * 128)
                        sc = sc_ps.tile([128, 1024], F32, tag="sc", name="sc")
                        # both heads' scores into one tile: two 64-contract
                        # row-tile matmuls (T0 / T8), one 1024-wide exp.
                        nc.tensor.matmul(sc[:, 0:512], kt_sb[t][pslA, ksl],
                                         qt[t][pslA, qsl], start=True, stop=True)
                        nc.tensor.matmul(sc[:, 512:1024], kt_sb[t][pslB, ksl],
                                         qt[t][pslB, qsl], start=True, stop=True)
                        e = e_pool.tile([128, 1024], MD, tag="e", name="e")
                        nc.scalar.activation(e[:], sc[:], AF.Exp, scale=0.125)
                        # PV runs one iteration behind: by emission time its
                        # exp has long completed, so the PV matmuls are never
                        # semaphore-gated and their weight-loads pipeline.
                        if pending is not None:
                            emit_pv(*pending)
                        pending = (e, kt)
                    emit_pv(*pending)
                    # normalize: all off ScalarE (see module docstring).
                    for pvp, psl in ((pvpA, pslA), (pvpB, pslB)):
                        uc = uc_pool.tile([HD + 1, 512], F32, tag="uc",
                                          name="uc")
                        nc.vector.tensor_copy(uc[:], pvp[:])
                        rcraw = rc_pool.tile([1, 512], F32, tag="rcraw",
                                             name="rcraw")
                        nc.sync.dma_start(rcraw[:], uc[HD:HD + 1, :])
                        rc = rc_pool.tile([1, 512], F32, tag="rc", name="rc")
                        nc.vector.reciprocal_approx_fast(rc[:], rcraw[:])
                        rcb = rcb_pool.tile([HD, 512], F32, tag="rcb",
                                            name="rcb")
                        nc.gpsimd.partition_broadcast(rcb[:], rc[:],
                                                      channels=HD)
                        nc.vector.tensor_tensor(ut[t][psl, qsl], uc[0:HD, :],
                                                rcb[:], mybir.AluOpType.mult)

                def weave_outproj(qq):
                    # out-projection of query quarter qq, 4 slices woven at
                    # kt 4/6/8/10 — late enough that the previous block's
                    # normalize chain is done before these hit the PE FIFO.
                    def w(kt):
                        if 4 <= kt < 12 and kt % 2 == 0:
                            out_proj(qq * 4 + (kt - 4) // 2)
                    return w

                ws = weave_sched
                attn_block(0, 0, weave_b00)
                attn_block(0, 1, ws({2: [("q0_2", 0)], 3: [("q0_2", 1)],
                                     8: [("k1_0", 0)], 9: [("k1_0", 1)]}))
                attn_block(0, 2, ws({2: [("q0_3", 0)], 3: [("q0_3", 1)],
                                     8: [("q1_0", 0)], 9: [("q1_0", 1)]}))
                attn_block(0, 3, ws({4: [("k1_1", 0)], 5: [("k1_1", 1)],
                                     10: [("k1_2", 0)], 11: [("k1_2", 1)]}))
                attn_block(1, 0, ws({0: [("k1_3", 0)], 1: [("k1_3", 1)],
                                     6: [("q1_1", 0)], 7: [("q1_1", 1)]}))
                attn_block(1, 1, lambda kt: (weave_outproj(0)(kt),
                                             ws({0: [("q1_2", 0)],
                                                 1: [("q1_2", 1)]})(kt)))
                attn_block(1, 2, lambda kt: (weave_outproj(1)(kt),
                                             ws({0: [("q1_3", 0)],
                                                 1: [("q1_3", 1)]})(kt)))
                attn_block(1, 3, weave_outproj(2))
                # PE filler during the final normalize chain: keeps the HAM
                # clock-gate warm so the tail out-projections run at full
                # clock.
                fps = sc_ps.tile([128, 1024], F32, tag="sc", name="sc")
                for _ in range(24):
                    nc.tensor.matmul(fps[:, 0:512], warm_sb[:, 0:128],
                                     warm_sb[:], start=True, stop=True,
                                     skip_group_check=True)
                for st in range(12, ST):
                    out_proj(st, evict_scalar=(st % 2 == 1))

    nc.compile()
    return nc


def _get_graph():
    if "nc" not in _CACHED:
        _CACHED["nc"] = build_core_graph()
    return _CACHED["nc"]


def _np_mode_dtype():
    if MODE == "bf16":
        import ml_dtypes
        return ml_dtypes.bfloat16
    return np.float32


def kernel(x, W_qkv, b_qkv, W_out, b_out):
    x = np.asarray(x, dtype=np.float32)
    W_qkv = np.asarray(W_qkv, dtype=np.float32)
    b_qkv = np.asarray(b_qkv, dtype=np.float32)
    W_out = np.asarray(W_out, dtype=np.float32)
    b_out = np.asarray(b_out, dtype=np.float32)

    nc = _get_graph()
    md = _np_mode_dtype()

    Wq, Wk, Wv = W_qkv[:, 0:E], W_qkv[:, E:2 * E], W_qkv[:, 2 * E:3 * E]
    bq, bk, bv = b_qkv[0:E], b_qkv[E:2 * E], b_qkv[2 * E:3 * E]

    in_maps = []
    for c in range(8):
        b, hg = c // 4, c % 4
        cols = slice(DQ * hg, DQ * hg + DQ)
        in_maps.append({
            "xt": np.ascontiguousarray(x[b].T).astype(md),
            "wq": np.ascontiguousarray(Wq[:, cols]).astype(md),
            "wk": np.ascontiguousarray(Wk[:, cols]).astype(md),
            "wv": np.ascontiguousarray(Wv[:, cols]).astype(md),
            "bq": np.ascontiguousarray(bq[cols].reshape(DQ, 1)),
            "bk": np.ascontiguousarray(bk[cols].reshape(DQ, 1)),
            "wo": np.ascontiguousarray(W_out[cols, :]).astype(md),
        })

    res = bass_utils.run_bass_kernel_spmd(nc, in_maps, core_ids=list(range(8)))
    _CACHED["last_results"] = res

    b_eff = (b_out.astype(np.float64) +
             bv.astype(np.float64) @ W_out.astype(np.float64))
    out = np.empty((2, S, E), np.float32)
    for b in range(2):
        acc = np.zeros((S, E), np.float64)
        for hg in range(4):
            acc += res.results[4 * b + hg]["out"].astype(np.float64)
        out[b] = (acc + b_eff).astype(np.float32)
    return out


# revision 36
# speedup vs baseline: 1.0115x; 1.0115x over previous
"""Distributed Trainium2 kernel for a 16-head self-attention block.

Model (reference):
    qkv = x @ W_qkv + b_qkv ; q,k,v per 16 heads (head_dim 64)
    attn = softmax(q k^T / 8) ; out = (attn @ v heads concat) @ W_out + b_out
Shapes: x [2, 2048, 1024], W_qkv [1024, 3072], W_out [1024, 1024].

Sharding (8 NeuronCores): 2 batch groups x 4 cores; each core owns one batch
element and 4 of the 16 heads (Megatron-style column-parallel QKV + row-
parallel out-proj). Each core computes its partial out-projection
[2048, 1024]; the host sums the 4 partials per batch (the Megatron
all-reduce, performed at unshard time) and adds the output bias.

Numerical notes:
  * softmax runs without max-subtraction: scores/8 here are bounded ~|2.5|.
  * the V bias folds out of attention exactly (softmax rows sum to 1), so
    bv@W_out is added to the host-side output bias instead of on device.

Device dataflow per core (no transposes anywhere):
  x^T [1024, 2048] (host pre-transposed, DMA'd in 512-column chunks so the
  first projection group starts ~5us in) ->
  Q^T,K^T [256, 2048] = Wq^T x^T (+bias on eviction);  V [2048, 4, 65]
    (per-head 64 dims + a ones column, which makes the PV matmul emit the
    softmax denominator as row 64 of U^T)
  attention in 8 blocks of (head-pair t, 512-query quarter qq):
    per 128-key tile kt ONE [128 k, 1024] PSUM tile holds both heads'
    scores^T (A cols 0:512, B cols 512:1024; the two 64-contract matmuls
    are row-tiles T0/T8), ONE 1024-wide exp on ScalarE -> E [128, 1024]
    bf16; pv [65, 512] per head += V[kt]^T E-half, emitted one kt behind
    so the PV matmuls are never semaphore-gated.
  The single exp per kt frees both score slots at once, so the next kt's
  pair of score matmuls gate on one event (sc ring bufs=3, 6 PSUM banks;
  pv pair 2 banks; 8 total).
  normalization (all off ScalarE — the old Ln/Exp route thrashed the
  activation-table sets): pv evicted to SBUF (DVE), denominator row DMA'd
  to a partition-0 tile (the custom-DVE reciprocal misreads nonzero input
  base partitions), reciprocal_approx_fast (DVE), GpSimd partition
  broadcast, one DVE multiply into ut (bf16).
  out partial [2048, 1024] bf16 = (U^T)^T @ Wo_rows, per 128-row slice.

Weave schedule (keeps ScalarE's exp stream as the pacer):
  pre-phase: K0 (all 4 qb) + Q0 qb0 only (~9us of PE), so the first exp
  fires ~15us in.
  block (t=0,qq=0): V projection (per-kt, just-in-time for PV) + Q0 qb1.
  blocks (0,1..3): K1/Q1 groups (1 per 3 kts) + Q0 qb2/qb3.
  blocks (1,1..3): out-projection of query quarter qq-1 (4 slices each,
  delayed to kt 4.. so the previous block's normalize chain never
  head-of-line blocks the PE FIFO).
  tail: PE filler matmuls bridge the last normalize chain (keeps the HAM
  clock-gate at 8/8), then the last quarter's out-projection.
"""

import contextlib
import os

import numpy as np

import concourse.bacc as bacc
import concourse.mybir as mybir
import concourse.tile as tile
from concourse import bass_utils

F32 = mybir.dt.float32
AF = mybir.ActivationFunctionType

S = 2048          # sequence length (one batch element per core)
E = 1024          # embedding dim
HD = 64           # head dim
NH = 4            # heads per core
DQ = NH * HD      # per-core q/k/v width (256)
ET = E // 128     # embedding tiles (8)
ST = S // 128     # sequence tiles (16)
QB = S // 512     # 512-wide query blocks (4)

MODE = os.environ.get("ATTN_MM_DTYPE", "bf16")  # bf16 | f32r | f32

_CACHED = {}


def build_core_graph(mode=MODE):
    MD = {"f32r": mybir.dt.float32r, "f32": F32, "bf16": mybir.dt.bfloat16}[mode]

    nc = bacc.Bacc("TRN2", target_bir_lowering=False, debug=False, num_devices=8)

    xt_d = nc.dram_tensor("xt", [E, S], MD, kind="ExternalInput")
    wq_d = nc.dram_tensor("wq", [E, DQ], MD, kind="ExternalInput")
    wk_d = nc.dram_tensor("wk", [E, DQ], MD, kind="ExternalInput")
    wv_d = nc.dram_tensor("wv", [E, DQ], MD, kind="ExternalInput")
    bq_d = nc.dram_tensor("bq", [DQ, 1], F32, kind="ExternalInput")
    bk_d = nc.dram_tensor("bk", [DQ, 1], F32, kind="ExternalInput")
    wo_d = nc.dram_tensor("wo", [DQ, E], MD, kind="ExternalInput")
    out_d = nc.dram_tensor("out", [S, E], mybir.dt.bfloat16, kind="ExternalOutput")

    with tile.TileContext(nc) as tc:
        with contextlib.ExitStack() as ctx:
            # ---- persistent SBUF pools ------------------------------------
            pers = ctx.enter_context(tc.tile_pool(name="pers", bufs=1))

            def ptile(shape, dtype, nm):
                return pers.tile(shape, dtype, tag=nm, name=nm)

            qt = [ptile([128, S], MD, f"qt{t}") for t in range(2)]
            ones_c = ptile([128, NH], F32, "ones_c")
            kt_sb = [ptile([128, S], MD, f"kt{t}") for t in range(2)]
            v_sb = [ptile([128, NH, HD + 1], MD, f"v{st}") for st in range(ST)]
            ut = [ptile([128, S], MD, f"ut{t}") for t in range(2)]
            wo_sb = [ptile([128, E], MD, f"wo{t}") for t in range(2)]
            bq_sb = [ptile([128, 1], F32, f"bq{t}") for t in range(2)]
            bk_sb = [ptile([128, 1], F32, f"bk{t}") for t in range(2)]

            nc.vector.memset(ones_c[:], 1.0)
            # dummy exp at t=0: hoists the one exp ACT_TABLE_LOAD into the
            # input-DMA window instead of right before the first real exp.
            dummy = ptile([1, 8], F32, "dummy")
            nc.vector.memset(dummy[:], 0.0)
            nc.scalar.activation(dummy[:], dummy[:], AF.Exp)
            # the ones column of every V tile is constant — write once here
            for st in range(ST):
                nc.vector.tensor_copy(v_sb[st][:, :, HD:HD + 1],
                                      ones_c[:].rearrange("p (h d) -> p h d", h=NH))
            for t in range(2):
                nc.scalar.dma_start(bq_sb[t][:], bq_d[t * 128:(t + 1) * 128, :])
                nc.scalar.dma_start(bk_sb[t][:], bk_d[t * 128:(t + 1) * 128, :])

            # x^T and weight tiles live until the last woven projection is
            # done. x^T is chunked per 512-query block so the first K
            # projection group only waits for ~1.5MB of input.
            apool = ctx.enter_context(tc.tile_pool(name="ab_sbuf", bufs=1))

            def atile(shape, nm):
                return apool.tile(shape, MD, tag=nm, name=nm)

            xtq = [[atile([128, 512], f"xt{et}_{qb}") for et in range(ET)]
                   for qb in range(QB)]
            wqs = [atile([128, DQ], f"wq{et}") for et in range(ET)]
            wks = [atile([128, DQ], f"wk{et}") for et in range(ET)]
            wvs = [atile([128, DQ], f"wv{et}") for et in range(ET)]
            # Input DMA: the projections consume x^T at ~590GB/s of demand,
            # so the head is input-bandwidth-bound. One issuing engine only
            # reaches its own few HWDGE queues (~130GB/s observed); stripe
            # every transfer round-robin across all three DMA-capable
            # engines (SP + ACT hwdge, Pool swdge) to engage the full queue
            # set. Order = first-consumption order.
            # The input demand curve (~2.5MB by 10us, +1MB per ~5us after)
            # exactly matches the measured ~250GB/s contended per-core HBM
            # rate, but ONLY if transfers complete in consumption order
            # globally. Per-engine queue FIFOs are imbalanced, so stripe one
            # priority-ordered list across the engines weighted by their
            # queue capacity (gpsimd SWDGE ~8 queues : sync ~6 : scalar ~4).
            def _sl(et):
                return slice(et * 128, (et + 1) * 128)

            dma_list = []
            for et in range(ET):
                dma_list.append((wks[et][:], wk_d[_sl(et), :]))
                dma_list.append((xtq[0][et][:], xt_d[_sl(et), 0:512]))
            for et in range(ET):
                dma_list.append((wqs[et][:], wq_d[_sl(et), :]))
            for et in range(ET):
                dma_list.append((wvs[et][:], wv_d[_sl(et), :]))
            for qb in range(1, QB):
                qsl = slice(qb * 512, (qb + 1) * 512)
                for et in range(ET):
                    dma_list.append((xtq[qb][et][:], xt_d[_sl(et), qsl]))
            for t in range(2):
                dma_list.append((wo_sb[t][:], wo_d[t * 128:(t + 1) * 128, :]))
            pat = [nc.gpsimd, nc.sync, nc.scalar, nc.gpsimd, nc.sync,
                   nc.gpsimd, nc.scalar, nc.sync, nc.gpsimd]
            for i, (dst, src) in enumerate(dma_list):
                pat[i % len(pat)].dma_start(dst, src)

            def proj_group(dst, wsrc, tsl, qb, bias, psum_pool):
                qsl = slice(qb * 512, (qb + 1) * 512)
                pp = psum_pool.tile([128, 512], F32, tag="sc", name="sc")
                for et in range(ET):
                    nc.tensor.matmul(pp[:], wsrc[et][:, tsl], xtq[qb][et][:],
                                     start=(et == 0), stop=(et == ET - 1))
                nc.vector.tensor_scalar_add(dst[:, qsl], pp[:], bias[:])

            def v_unit(kt, psum_pool):
                # V projection for one 128-row sequence tile: reuses the
                # x^T chunks already resident for the K/Q groups.
                ssl = slice((kt % 4) * 128, (kt % 4 + 1) * 128)
                pv = psum_pool.tile([128, DQ], F32, tag="sc", name="sc")
                for et in range(ET):
                    nc.tensor.matmul(pv[:], xtq[kt // 4][et][:, ssl],
                                     wvs[et][:],
                                     start=(et == 0), stop=(et == ET - 1))
                nc.vector.tensor_copy(v_sb[kt][:, :, 0:HD],
                                      pv[:].rearrange("p (h d) -> p h d", h=NH))

            # ---- stage A-pre: warmup + K0 qb0 + Q0 qb0 + V 0..3 -----------
            # Minimal: just enough for block (0,0)'s first 4 key tiles. The
            # rest of K0 and V weave into block (0,0) just-in-time as their
            # x^T chunks land (input streaming is ~250GB/s, so the head is
            # bandwidth-gated; the exp stream must start as early as
            # possible).
            with tc.tile_pool(name="a_ps", bufs=5, space="PSUM") as a_ps:
                # HAM warm-up: keep the PE busy during the input-DMA window
                # so the clock-gate releases (K=8/8) before the real matmul
                # stream begins. Garbage data, never read.
                warm_sb = apool.tile([128, 512], MD, tag="warm", name="warm")
                nc.vector.memset(warm_sb[:], 0.0)
                wps = a_ps.tile([128, 512], F32, tag="wps", name="wps", bufs=1)
                for _ in range(12):
                    nc.tensor.matmul(wps[:], warm_sb[:, 0:128], warm_sb[:],
                                     start=True, stop=True)

                proj_group(kt_sb[0], wks, slice(0, 128), 0, bk_sb[0], a_ps)
                proj_group(qt[0], wqs, slice(0, 128), 0, bq_sb[0], a_ps)
                for kt in range(4):
                    v_unit(kt, a_ps)

            # ---- stage B: attention in 8 (head-pair, query-quarter) blocks
            with tc.tile_pool(name="e_sb", bufs=6) as e_pool, \
                 tc.tile_pool(name="uc_sb", bufs=4) as uc_pool, \
                 tc.tile_pool(name="rc_sb", bufs=4) as rc_pool, \
                 tc.tile_pool(name="rcb_sb", bufs=2) as rcb_pool, \
                 tc.tile_pool(name="o_sb", bufs=3) as o_pool, \
                 tc.tile_pool(name="sc_ps", bufs=3, space="PSUM") as sc_ps, \
                 tc.tile_pool(name="pv_ps", bufs=1, space="PSUM") as pv_ps:

                def out_proj(st, evict_scalar=False):
                    ssl = slice(st * 128, (st + 1) * 128)
                    op = sc_ps.tile([128, E], F32, tag="sc", name="sc")
                    for ob in range(2):
                        osl = slice(ob * 512, (ob + 1) * 512)
                        for t2 in range(2):
                            nc.tensor.matmul(op[:, osl], ut[t2][:, ssl],
                                             wo_sb[t2][:, osl],
                                             start=(t2 == 0), stop=(t2 == 1))
                    o_sb = o_pool.tile([128, E], mybir.dt.bfloat16, tag="o", name="o")
                    if evict_scalar:
                        # tail-only: ScalarE is idle there, and alternating
                        # the evictions doubles PSUM-drain throughput
                        nc.scalar.activation(o_sb[:], op[:], AF.Copy)
                    else:
                        nc.vector.tensor_copy(o_sb[:], op[:])
                    # two half-width DMAs land on different queues: halves
                    # the post-eviction transfer latency on the tail slices.
                    nc.sync.dma_start(out_d[ssl, 0:512], o_sb[:, 0:512])
                    nc.sync.dma_start(out_d[ssl, 512:1024], o_sb[:, 512:1024])

                # block (0,0) weave: the rest of K0 and V, just-in-time.
                # V(j) is consumed by emit_pv(j), which executes at kt j+1;
                # weaving V(j) at kt j is safe (the weave precedes scores
                # and the lagging PV in the PE FIFO). K0 qb g is first read
                # by scores kt=4g; woven 3-4 kts ahead. Both match the x^T
                # chunk arrival order.
                # kt -> half-group; V units fill the rest just-in-time.
                b00_k0 = {1: ("k0_1", 0), 2: ("k0_1", 1),
                          5: ("k0_2", 0), 6: ("k0_2", 1),
                          9: ("k0_3", 0), 10: ("k0_3", 1)}

                def weave_b00(kt):
                    if kt == 0:
                        proj_group(qt[0], wqs, slice(0, 128), 1, bq_sb[0],
                                   sc_ps)
                    g = b00_k0.get(kt)
                    if g is not None:
                        proj_half(*g)
                    if 2 <= kt <= 4:
                        v_unit(kt + 2, sc_ps)
                    elif 6 <= kt <= 8:
                        v_unit(kt + 1, sc_ps)
                    elif kt >= 10:
                        v_unit(kt, sc_ps)

                # remaining projection groups, identified by name, woven into
                # specific (block, kt) slots below. Deadlines: Q0qbN before
                # block (0,N); K1qb0+Q1qb0 before block (1,0); K1qbN before
                # block (1,0) kt 4N; Q1qbN before block (1,N).
                G = {}
                for qb in range(1, QB):
                    G[f"q0_{qb}"] = (qt[0], wqs, slice(0, 128), qb, bq_sb[0])
                    G[f"k0_{qb}"] = (kt_sb[0], wks, slice(0, 128), qb,
                                     bk_sb[0])
                for qb in range(QB):
                    G[f"k1_{qb}"] = (kt_sb[1], wks, slice(128, 256), qb,
                                     bk_sb[1])
                    G[f"q1_{qb}"] = (qt[1], wqs, slice(128, 256), qb,
                                     bq_sb[1])

                held = {}

                def proj_half(gname, half):
                    # half a projection group (4 of 8 contract tiles): keeps
                    # the per-kt PE overload under ~0.9us so the exp stream
                    # bubbles stay small. The PSUM accumulator tile is held
                    # across the two halves (intervening matmuls hit other
                    # banks — legal).
                    dst, wsrc, tsl, qb, bias = G[gname]
                    qsl = slice(qb * 512, (qb + 1) * 512)
                    if half == 0:
                        pp = sc_ps.tile([128, 512], F32, tag="sc", name="sc")
                        held[gname] = pp
                        for et in range(4):
                            nc.tensor.matmul(pp[:], wsrc[et][:, tsl],
                                             xtq[qb][et][:],
                                             start=(et == 0), stop=False)
                    else:
                        pp = held.pop(gname)
                        for et in range(4, ET):
                            nc.tensor.matmul(pp[:], wsrc[et][:, tsl],
                                             xtq[qb][et][:],
                                             start=False, stop=(et == ET - 1))
                        nc.vector.tensor_scalar_add(dst[:, qsl], pp[:],
                                                    bias[:])

                def weave_sched(sched):
                    def w(kt):
                        for gname, half in sched.get(kt, ()):
                            proj_half(gname, half)
                    return w

                def attn_block(t, qq, weave):
                    hA, hB = 2 * t, 2 * t + 1
                    pslA, pslB = slice(0, 64), slice(64, 128)
                    qsl = slice(qq * 512, (qq + 1) * 512)
                    pvpA = pv_ps.tile([HD + 1, 512], F32, tag="pvA", name="pvA")
                    pvpB = pv_ps.tile([HD + 1, 512], F32, tag="pvB", name="pvB")

                    def emit_pv(e, kt):
                        nc.tensor.matmul(pvpA[:], v_sb[kt][:, hA, :],
                                         e[:, 0:512],
                                         start=(kt == 0), stop=(kt == ST - 1))
                        nc.tensor.matmul(pvpB[:], v_sb[kt][:, hB, :],
                                         e[:, 512:1024],
                                         start=(kt == 0), stop=(kt == ST - 1))

                    pending = None
                    for kt in range(ST):
                        if weave is not None:
                            weave(kt)
                        ksl = slice(kt * 128, (kt + 1) * 128)
                        sc = sc_ps.tile([128, 1024], F32, tag="sc", name="sc")
                        # both heads' scores into one tile: two 64-contract
                        # row-tile matmuls (T0 / T8), one 1024-wide exp.
                        nc.tensor.matmul(sc[:, 0:512], kt_sb[t][pslA, ksl],
                                         qt[t][pslA, qsl], start=True, stop=True)
                        nc.tensor.matmul(sc[:, 512:1024], kt_sb[t][pslB, ksl],
                                         qt[t][pslB, qsl], start=True, stop=True)
                        e = e_pool.tile([128, 1024], MD, tag="e", name="e")
                        nc.scalar.activation(e[:], sc[:], AF.Exp, scale=0.125)
                        # PV runs one iteration behind: by emission time its
                        # exp has long completed, so the PV matmuls are never
                        # semaphore-gated and their weight-loads pipeline.
                        if pending is not None:
                            emit_pv(*pending)
                        pending = (e, kt)
                    emit_pv(*pending)
                    # normalize: all off ScalarE (see module docstring).
                    for pvp, psl in ((pvpA, pslA), (pvpB, pslB)):
                        uc = uc_pool.tile([HD + 1, 512], F32, tag="uc",
                                          name="uc")
                        nc.vector.tensor_copy(uc[:], pvp[:])
                        rcraw = rc_pool.tile([1, 512], F32, tag="rcraw",
                                             name="rcraw")
                        nc.sync.dma_start(rcraw[:], uc[HD:HD + 1, :])
                        rc = rc_pool.tile([1, 512], F32, tag="rc", name="rc")
                        nc.vector.reciprocal_approx_fast(rc[:], rcraw[:])
                        rcb = rcb_pool.tile([HD, 512], F32, tag="rcb",
                                            name="rcb")
                        nc.gpsimd.partition_broadcast(rcb[:], rc[:],
                                                      channels=HD)
                        nc.vector.tensor_tensor(ut[t][psl, qsl], uc[0:HD, :],
                                                rcb[:], mybir.AluOpType.mult)

                def weave_outproj(qq):
                    # out-projection of query quarter qq, 4 slices woven at
                    # kt 4/6/8/10 — late enough that the previous block's
                    # normalize chain is done before these hit the PE FIFO.
                    def w(kt):
                        if 4 <= kt < 12 and kt % 2 == 0:
                            out_proj(qq * 4 + (kt - 4) // 2)
                    return w

                ws = weave_sched
                attn_block(0, 0, weave_b00)
                attn_block(0, 1, ws({2: [("q0_2", 0)], 3: [("q0_2", 1)],
                                     8: [("k1_0", 0)], 9: [("k1_0", 1)]}))
                attn_block(0, 2, ws({2: [("q0_3", 0)], 3: [("q0_3", 1)],
                                     8: [("q1_0", 0)], 9: [("q1_0", 1)]}))
                attn_block(0, 3, ws({4: [("k1_1", 0)], 5: [("k1_1", 1)],
                                     10: [("k1_2", 0)], 11: [("k1_2", 1)]}))
                attn_block(1, 0, ws({0: [("k1_3", 0)], 1: [("k1_3", 1)],
                                     6: [("q1_1", 0)], 7: [("q1_1", 1)]}))
                attn_block(1, 1, lambda kt: (weave_outproj(0)(kt),
                                             ws({0: [("q1_2", 0)],
                                                 1: [("q1_2", 1)]})(kt)))
                attn_block(1, 2, lambda kt: (weave_outproj(1)(kt),
                                             ws({0: [("q1_3", 0)],
                                                 1: [("q1_3", 1)]})(kt)))
                attn_block(1, 3, weave_outproj(2))
                # PE filler during the final normalize chain: keeps the HAM
                # clock-gate warm so the tail out-projections run at full
                # clock.
                fps = sc_ps.tile([128, 1024], F32, tag="sc", name="sc")
                for _ in range(24):
                    nc.tensor.matmul(fps[:, 0:512], warm_sb[:, 0:128],
                                     warm_sb[:], start=True, stop=True,
                                     skip_group_check=True)
                for st in range(12, ST):
                    out_proj(st, evict_scalar=(st % 2 == 1))

    nc.compile()
    return nc


def _get_graph():
    if "nc" not in _CACHED:
        _CACHED["nc"] = build_core_graph()
    return _CACHED["nc"]


def _np_mode_dtype():
    if MODE == "bf16":
        import ml_dtypes
        return ml_dtypes.bfloat16
    return np.float32


def kernel(x, W_qkv, b_qkv, W_out, b_out):
    x = np.asarray(x, dtype=np.float32)
    W_qkv = np.asarray(W_qkv, dtype=np.float32)
    b_qkv = np.asarray(b_qkv, dtype=np.float32)
    W_out = np.asarray(W_out, dtype=np.float32)
    b_out = np.asarray(b_out, dtype=np.float32)

    nc = _get_graph()
    md = _np_mode_dtype()

    Wq, Wk, Wv = W_qkv[:, 0:E], W_qkv[:, E:2 * E], W_qkv[:, 2 * E:3 * E]
    bq, bk, bv = b_qkv[0:E], b_qkv[E:2 * E], b_qkv[2 * E:3 * E]

    in_maps = []
    for c in range(8):
        b, hg = c // 4, c % 4
        cols = slice(DQ * hg, DQ * hg + DQ)
        in_maps.append({
            "xt": np.ascontiguousarray(x[b].T).astype(md),
            "wq": np.ascontiguousarray(Wq[:, cols]).astype(md),
            "wk": np.ascontiguousarray(Wk[:, cols]).astype(md),
            "wv": np.ascontiguousarray(Wv[:, cols]).astype(md),
            "bq": np.ascontiguousarray(bq[cols].reshape(DQ, 1)),
            "bk": np.ascontiguousarray(bk[cols].reshape(DQ, 1)),
            "wo": np.ascontiguousarray(W_out[cols, :]).astype(md),
        })

    res = bass_utils.run_bass_kernel_spmd(nc, in_maps, core_ids=list(range(8)))
    _CACHED["last_results"] = res

    b_eff = (b_out.astype(np.float64) +
             bv.astype(np.float64) @ W_out.astype(np.float64))
    out = np.empty((2, S, E), np.float32)
    for b in range(2):
        acc = np.zeros((S, E), np.float64)
        for hg in range(4):
            acc += res.results[4 * b + hg]["out"].astype(np.float64)
        out[b] = (acc + b_eff).astype(np.float32)
    return out
